# revision 1
# baseline (speedup 1.0000x reference)
"""CenterLoss kernel for 8 Trainium2 NeuronCores.

Math: with d=DECAY, e=1-d, per-class mean m_c = s_c/n_c (s_c = sum of batch
features of class c, n_c = count), the reference loss decomposes exactly:

  loss*B*F = sum_i ||f_i - d*c_{l_i} - e*m_{l_i}||^2
           = alpha + d^2*gamma - 2*d^2*beta - e*(2-e)*Q
  alpha = sum_i ||f_i||^2
  beta  = sum_i f_i . c_{l_i}
  gamma = sum_i ||c_{l_i}||^2
  Q     = sum_c ||s_c||^2 / n_c
        = sum_i ||f_i||^2/n_{l_i}  +  sum_{same-class pairs i<j} 2 f_i.f_j / n_c

Every term is gather + elementwise + reduce: no scatter needed. The host only
routes labels (sort, bincount, pair lists, class-range partitioning for the
sharded center table); all feature/center data movement and arithmetic runs
on the 8 cores. Per core: its 2048 sorted samples, a class-range slice of the
center table, and its same-class pair list. alpha and the Q1 term fold into
one weighted sum P0 = sum_i w'_i*||f_i||^2 with w'_i = 1 - e*(2-e)/n_i,
applied as a per-sample sqrt(w') scale inside the ACT Square instruction.
"""

import os
import sys

import numpy as np

for _p in ("/opt/trn_rl_repo",):
    if _p not in sys.path and os.path.isdir(_p):
        sys.path.insert(0, _p)

B = 16384
F = 256
C = 100000
DECAY = 0.99
NCORES = 8

T = B // NCORES          # samples per core (exact split of sorted order)
NT = T // 128            # feature blocks of [128, F] per core
CT = 16384               # padded class-table rows per core (max class span)
PT = 2                   # pair blocks per core
NP = PT * 128            # device pair capacity per core
HOST_PAIR_LIMIT = 100000  # beyond this, fall back to full host compute

_E = 1.0 - DECAY
_QCOEF = _E * (2.0 - _E)          # 0.0199
_D2 = DECAY * DECAY               # 0.9801

_nc_cache = None
_LAST_RESULT = None


def _ensure_ntff_hook():
    """bass_utils' trace path does `from antenv.axon_hooks import ...`
    unconditionally; some agent images lack that module. Register a stub
    (and wire the real ctypes NTFF hook when available) so trace=True /
    BASS_TRACE=1 degrades gracefully instead of crashing."""
    try:
        import antenv.axon_hooks  # noqa: F401
        return
    except ImportError:
        pass
    import types

    try:
        import antenv
    except ImportError:
        return
    mod = types.ModuleType("antenv.axon_hooks")
    holder = {"h": None}
    mod.set_axon_ntff_profile_hook = lambda h: holder.__setitem__("h", h)
    mod.get_axon_ntff_profile_hook = lambda: holder["h"]
    sys.modules["antenv.axon_hooks"] = mod
    antenv.axon_hooks = mod
    try:
        import importlib.util

        so = "/opt/axon/libaxon_pjrt.so"
        boot_py = "/root/.axon_site/trn_agent_boot/trn_boot.py"
        if os.path.exists(so) and os.path.exists(boot_py):
            spec = importlib.util.spec_from_file_location("_trn_boot_hookmod", boot_py)
            tb = importlib.util.module_from_spec(spec)
            spec.loader.exec_module(tb)
            h = tb._ntff_profile_via_ctypes(so)
            if h is not None:
                mod.set_axon_ntff_profile_hook(h)
    except Exception:
        pass


def _build_bass():
    import concourse.mybir as mybir
    import concourse.tile as tile
    from concourse import bacc

    f32 = mybir.dt.float32
    i16 = mybir.dt.int16

    # Bacc (not raw Bass): its finalize() legalizes multi-wait sync into
    # event-semaphore instructions and runs register allocation — this
    # walrus rejects >1 sync wait per instruction otherwise.
    nc = bacc.Bacc(None)
    feat = nc.dram_tensor("feat", [T, F], f32, kind="ExternalInput")
    ctab = nc.dram_tensor("ctab", [CT, F], f32, kind="ExternalInput")
    cidx = nc.dram_tensor("cidx", [128, T // 16], i16, kind="ExternalInput")
    swin = nc.dram_tensor("sw", [128, NT], f32, kind="ExternalInput")
    pidxin = nc.dram_tensor("pidx", [128, 2 * NP // 16], i16, kind="ExternalInput")

    GCH = 4                    # center gather / feat-load chunks
    SLOTS_PER_CH = T // GCH    # gather slots per chunk
    BLK_PER_CH = NT // GCH     # feature blocks per chunk

    out_aq = nc.dram_tensor("aq", [128, NT], f32, kind="ExternalOutput")
    out_bb = nc.dram_tensor("bb", [128, NT], f32, kind="ExternalOutput")
    out_gg = nc.dram_tensor("gg", [128, GCH], f32, kind="ExternalOutput")
    out_qq = nc.dram_tensor("qq", [128, PT], f32, kind="ExternalOutput")

    from concourse.tile import add_dep_helper

    with tile.TileContext(nc) as tc:
        with (
            tc.tile_pool(name="io", bufs=1) as io,
            tc.tile_pool(name="acc", bufs=1) as acc,
            tc.tile_pool(name="ascr", bufs=3) as ascr,
            tc.tile_pool(name="vscr", bufs=3) as vscr,
        ):
            # Index loads first: the drain that precedes the first SWDGE
            # gather waits on all outstanding DMAs, so the big feature DMA
            # is issued after the gathers are underway.
            cidx_t = io.tile([128, T // 16], dtype=i16)
            nc.sync.dma_start(cidx_t[:], cidx[:, :])
            pidx_t = io.tile([128, 2 * NP // 16], dtype=i16)
            nc.sync.dma_start(pidx_t[:], pidxin[:, :])

            # Centers: gather slot s -> (partition s%128, block s//128); the
            # host orders cidx so slot s carries sample (s%128)*NT + s//128,
            # aligning gathered rows with the feat layout.
            call = io.tile([128, NT * F], dtype=f32)
            call3 = call[:].rearrange("p (n d) -> p n d", d=F)
            pp = io.tile([128, 2 * PT * F], dtype=f32)
            gather_insts = []
            for g in range(GCH):
                gi = nc.gpsimd.dma_gather(
                    call3[:, g * BLK_PER_CH:(g + 1) * BLK_PER_CH, :],
                    ctab[:, :],
                    cidx_t[:, g * (SLOTS_PER_CH // 16):(g + 1) * (SLOTS_PER_CH // 16)],
                    SLOTS_PER_CH,
                    SLOTS_PER_CH,
                    F,
                )
                gather_insts.append(gi)

            # Pair rows (one gather: left rows in the first PT blocks,
            # right rows in the next PT).
            nc.gpsimd.dma_gather(
                pp[:].rearrange("p (n d) -> p n d", d=F), feat[:, :],
                pidx_t[:], 2 * NP, 2 * NP, F)

            sw_t = io.tile([128, NT], dtype=f32)
            sw_dma = nc.sync.dma_start(sw_t[:], swin[:, :])
            add_dep_helper(sw_dma.ins, gather_insts[0].ins, sync=True)
            # Features: partition p holds rows p*NT .. p*NT+NT-1 (contiguous
            # 16 KiB per partition -> large DMA descriptors), chunked, and
            # held back until the first gather completes so the dge_drain
            # preceding the SWDGE gathers never waits on them.
            fall = io.tile([128, NT * F], dtype=f32)
            featr = feat.rearrange("(p n) d -> p (n d)", p=128)
            for g in range(GCH):
                fd = nc.sync.dma_start(
                    fall[:, g * BLK_PER_CH * F:(g + 1) * BLK_PER_CH * F],
                    featr[:, g * BLK_PER_CH * F:(g + 1) * BLK_PER_CH * F])
                add_dep_helper(fd.ins, gather_insts[0].ins, sync=True)

            aq = acc.tile([128, NT], dtype=f32)
            bb = acc.tile([128, NT], dtype=f32)
            gg = acc.tile([128, GCH], dtype=f32)
            qq = acc.tile([128, PT], dtype=f32)

            # ACT: per-block f Square with per-sample scale (folds the w'
            # weighting) and free-axis accumulate.
            for t in range(NT):
                fb = fall[:, t * F:(t + 1) * F]
                a_scr = ascr.tile([128, F], dtype=f32, tag="ascr")
                nc.scalar.activation(
                    a_scr[:], fb, mybir.ActivationFunctionType.Square,
                    scale=sw_t[:, t:t + 1], accum_out=aq[:, t:t + 1])

            # ACT: gamma as one accumulated Square per gather chunk.
            for g in range(GCH):
                cb = call[:, g * BLK_PER_CH * F:(g + 1) * BLK_PER_CH * F]
                a_scr2 = ascr.tile([128, BLK_PER_CH * F], dtype=f32, tag="gscr")
                nc.scalar.activation(
                    a_scr2[:], cb, mybir.ActivationFunctionType.Square,
                    accum_out=gg[:, g:g + 1])

            # DVE: f.c row dots, chunked to overlap with the center gather.
            # (tensor_tensor_reduce is rejected by this runtime; use plain
            # tensor_tensor + tensor_reduce.)
            last_red = None
            for g in range(GCH):
                lo, hi = g * BLK_PER_CH, (g + 1) * BLK_PER_CH
                v_scr = vscr.tile([128, BLK_PER_CH * F], dtype=f32, tag="vscr")
                nc.vector.tensor_tensor(
                    out=v_scr[:], in0=fall[:, lo * F:hi * F],
                    in1=call[:, lo * F:hi * F], op=mybir.AluOpType.mult)
                last_red = nc.vector.tensor_reduce(
                    out=bb[:, lo:hi],
                    in_=v_scr[:].rearrange("p (n d) -> p n d", d=F),
                    axis=mybir.AxisListType.X, op=mybir.AluOpType.add)

            # Pair row dots (unweighted; the host applies 2/n per slot).
            # Ordered after the center dots so the DVE stream isn't blocked
            # on the pair gathers (which run last on the Pool engine).
            p_scr = vscr.tile([128, PT * F], dtype=f32, tag="pscr")
            ptt = nc.vector.tensor_tensor(
                out=p_scr[:], in0=pp[:, :PT * F], in1=pp[:, PT * F:],
                op=mybir.AluOpType.mult)
            add_dep_helper(ptt.ins, last_red.ins, sync=False)
            nc.vector.tensor_reduce(
                out=qq[:, :],
                in_=p_scr[:].rearrange("p (n d) -> p n d", d=F),
                axis=mybir.AxisListType.X, op=mybir.AluOpType.add)

            nc.sync.dma_start(out_aq[:, :], aq[:])
            nc.sync.dma_start(out_bb[:, :], bb[:])
            nc.sync.dma_start(out_gg[:, :], gg[:])
            nc.sync.dma_start(out_qq[:, :], qq[:])
    nc.finalize()
    return nc


def _get_nc():
    global _nc_cache
    if _nc_cache is None:
        _nc_cache = _build_bass()
    return _nc_cache


def _wrap16(idx, n):
    """Lay out gather indices the way InstDMAGatherAnt consumes them:
    index j lives at [j % 16, j // 16], replicated to all 8 GPSIMD-core
    partition groups of a [128, n//16] int16 tile."""
    w = np.asarray(idx, dtype=np.int16).reshape(n // 16, 16).T
    return np.ascontiguousarray(np.tile(w, (8, 1)))


def _host_reference(f, labels, cf):
    """Full-precision host fallback (pathological label distributions only)."""
    f64 = f.astype(np.float64)
    sums = np.zeros((C, F), np.float64)
    np.add.at(sums, labels, f64)
    counts = np.bincount(labels, minlength=C).astype(np.float64)
    mean = sums / np.maximum(counts, 1.0)[:, None]
    newc = np.where((counts > 0)[:, None],
                    DECAY * cf.astype(np.float64) + (1 - DECAY) * mean,
                    cf.astype(np.float64))
    g = newc[labels]
    return np.float32(np.mean((f64 - g) ** 2))


def kernel(batch_feature, batch_label, center_feature):
    global _LAST_RESULT
    f = np.ascontiguousarray(np.asarray(batch_feature, dtype=np.float32))
    labels = np.asarray(batch_label).astype(np.int64)
    cf = np.ascontiguousarray(np.asarray(center_feature, dtype=np.float32))

    order = np.argsort(labels, kind="stable")
    sl = labels[order]                       # sorted labels
    uniq, run_start, run_cnt = np.unique(sl, return_index=True,
                                         return_counts=True)
    cnt_sorted = np.repeat(run_cnt, run_cnt)  # class count per sorted sample
    wq = 1.0 - _QCOEF / cnt_sorted            # w' per sorted sample
    sw = np.sqrt(wq).astype(np.float32)

    # Same-class pairs (global sorted positions). For uniform-random labels
    # this is ~1.3k pairs; a pathological distribution overflows to the host.
    dup = np.nonzero(run_cnt >= 2)[0]
    n_pairs_total = int(((run_cnt * (run_cnt - 1)) // 2).sum())
    if n_pairs_total > HOST_PAIR_LIMIT:
        return _host_reference(f, labels, cf)

    dev_pairs = [[] for _ in range(NCORES)]   # (local_i, local_j, 2/n)
    host_pairs = []                           # (global_pos_i, global_pos_j, 2/n)
    for r in dup:
        s0, n = int(run_start[r]), int(run_cnt[r])
        wgt = 2.0 / n
        for a in range(s0, s0 + n):
            for b in range(a + 1, s0 + n):
                ka, kb = a // T, b // T
                if ka == kb and len(dev_pairs[ka]) < NP:
                    dev_pairs[ka].append((a - ka * T, b - kb * T, wgt))
                else:
                    host_pairs.append((a, b, wgt))

    # Per-core class-range slices of the center table.
    in_maps = []
    pw_list = []
    fallback = False
    for k in range(NCORES):
        seg = slice(k * T, (k + 1) * T)
        rows = order[seg]
        sl_k = sl[seg]
        cls_lo = int(sl_k[0])
        span = int(sl_k[-1]) - cls_lo + 1
        if span > CT:
            fallback = True
            break
        ctab_k = np.zeros((CT, F), np.float32)
        ctab_k[:span] = cf[cls_lo:cls_lo + span]
        rebased = (sl_k - cls_lo).astype(np.int16)

        # center gather slot s carries sample (s%128)*NT + s//128
        sigma = (np.arange(T) % 128) * NT + (np.arange(T) // 128)
        cidx_k = _wrap16(rebased[sigma], T)
        sw_k = sw[seg].reshape(128, NT)

        plv = np.zeros(NP, np.int64)
        prv = np.zeros(NP, np.int64)
        pwv = np.zeros(NP, np.float64)
        for s, (ia, jb, wgt) in enumerate(dev_pairs[k]):
            plv[s], prv[s], pwv[s] = ia, jb, wgt
        pw_k = np.zeros((128, PT), np.float64)
        pw_k[np.arange(NP) % 128, np.arange(NP) // 128] = pwv
        pw_list.append(pw_k)

        in_maps.append({
            "feat": f[rows],
            "ctab": ctab_k,
            "cidx": cidx_k,
            "sw": np.ascontiguousarray(sw_k, dtype=np.float32),
            "pidx": _wrap16(np.concatenate([plv, prv]), 2 * NP),
        })
    if fallback:
        return _host_reference(f, labels, cf)

    _ensure_ntff_hook()
    from concourse.bass_utils import run_bass_kernel_spmd

    nc = _get_nc()
    res = run_bass_kernel_spmd(nc, in_maps, core_ids=list(range(NCORES)))
    _LAST_RESULT = res

    p0 = beta = gamma = q2 = 0.0
    for k, r in enumerate(res.results):
        p0 += float(np.asarray(r["aq"], np.float64).sum())
        beta += float(np.asarray(r["bb"], np.float64).sum())
        gamma += float(np.asarray(r["gg"], np.float64).sum())
        q2 += float((pw_list[k] * np.asarray(r["qq"], np.float64)).sum())

    for a, b, wgt in host_pairs:
        q2 += wgt * float(np.dot(f[order[a]].astype(np.float64),
                                 f[order[b]].astype(np.float64)))

    loss = (p0 + _D2 * (gamma - 2.0 * beta) - _QCOEF * q2) / (B * F)
    return np.float32(loss)

